# revision 8
# baseline (speedup 1.0000x reference)
"""APPNP (GNN message passing) on 8 Trainium2 NeuronCores.

Sharding (follows the hint): nodes and their segment-sums are sharded
across the 8 cores by node id (12500 each, edges partitioned by dst); the
MLP weights are replicated; each hop AllGathers every core's refreshed
hn = h*norm shard into a full per-core DRAM table, and each core gathers
hn[src] for its dst-sorted edge slots with indirect (per-partition) DMA,
accumulating on the vector engine.

Per-core layout: nodes are permuted by ascending in-degree ("rank"),
rank = chunk*128 + lane over 98 chunks. Chunk c pads every node to the
chunk max in-degree ghat[c] (cross-core max so the SPMD program is
uniform). Chunks are processed in 3 degree bands; each band runs one
Tile For_i loop over slot index j, whose body copies the j-th index
column for the band's chunks and issues one 128-descriptor indirect
gather plus one accumulate per chunk.

Host preprocessing is pure index manipulation (degree counts, sorting,
slot tables); all float math runs on device.
"""

import numpy as np

import concourse.bass as bass
import concourse.mybir as mybir
from concourse import bacc
from concourse.bass import IndirectOffsetOnAxis
from concourse.bass_utils import run_bass_kernel_spmd
from concourse.tile import TileContext

N_NODES = 100000
N_EDGES = 3200000
IN_F = 256
HID = 128
NC = 16
ALPHA = 0.1
K_HOPS = 10

M = 8                   # cores
CORE_N = N_NODES // M   # 12500 real nodes per core
P = 128                 # partitions / lanes
NCHUNK = 98             # chunks of 128 ranks
SHARD = NCHUNK * P      # 12544 ranks per core (44 dummies)
SHARD_T = SHARD + 1     # table stride per core: one extra all-zero row
SENT = SHARD            # sentinel row (core 0's zero row)
TAB = M * SHARD_T       # table rows
BANDS = [(0, 33), (33, 66), (66, 98)]   # chunk ranges per degree band

F32 = mybir.dt.float32
I32 = mybir.dt.int32


def _preprocess(src, dst):
    """Pure index-space preprocessing. Returns per-core tables + metadata."""
    src = np.asarray(src).astype(np.int64)
    dst = np.asarray(dst).astype(np.int64)
    deg = np.bincount(dst, minlength=N_NODES).astype(np.int64)

    trow = np.empty(N_NODES, dtype=np.int64)
    node_of_rank = np.empty((M, SHARD), dtype=np.int64)
    cnt_lane = np.zeros((M, P, NCHUNK), dtype=np.int32)
    gmax = np.zeros((M, NCHUNK), dtype=np.int64)

    for m in range(M):
        base = m * CORE_N
        d = deg[base:base + CORE_N]
        dpad = np.concatenate([d, np.full(SHARD - CORE_N, -1, dtype=np.int64)])
        order = np.argsort(dpad, kind="stable")   # ascending; dummies first
        real = order < CORE_N
        node_of_rank[m] = np.where(real, base + order, -1)
        trow[base + order[real]] = m * SHARD_T + np.flatnonzero(real)
        dr = np.where(real, dpad[order], 0)
        cnt_lane[m] = dr.reshape(NCHUNK, P).T
        gmax[m] = dr.reshape(NCHUNK, P).max(axis=1)

    ghat = np.maximum(gmax.max(axis=0), 1).astype(np.int64)  # per chunk
    # band height = max ghat inside the band; idx layout is j-major per band
    hband = [int(ghat[b0:b1].max()) for b0, b1 in BANDS]
    boff = np.zeros(len(BANDS) + 1, dtype=np.int64)
    for bi, ((b0, b1), h) in enumerate(zip(BANDS, hband)):
        boff[bi + 1] = boff[bi] + h * (b1 - b0)
    slot_p = int(boff[-1])

    band_of_chunk = np.empty(NCHUNK, dtype=np.int64)
    for bi, (b0, b1) in enumerate(BANDS):
        band_of_chunk[b0:b1] = bi
    boff_c = boff[band_of_chunk]
    nb_c = np.array([BANDS[band_of_chunk[c]][1] - BANDS[band_of_chunk[c]][0]
                     for c in range(NCHUNK)], dtype=np.int64)
    c0_c = np.array([BANDS[band_of_chunk[c]][0] for c in range(NCHUNK)],
                    dtype=np.int64)

    gidx = np.full((M, P, slot_p), SENT, dtype=np.int32)
    core_of_dst = dst // CORE_N
    for m in range(M):
        mask = core_of_dst == m
        s_e = src[mask]
        rho = trow[dst[mask]] - m * SHARD_T
        o2 = np.argsort(rho, kind="stable")
        rho_s = rho[o2]
        src_s = s_e[o2]
        n_e = len(rho_s)
        if n_e == 0:
            continue
        first = np.r_[0, np.flatnonzero(np.diff(rho_s)) + 1]
        run_len = np.diff(np.r_[first, n_e])
        j = np.arange(n_e) - np.repeat(first, run_len)
        c = rho_s // P
        p = rho_s % P
        col = boff_c[c] + j * nb_c[c] + (c - c0_c[c])
        assert (j < ghat[c]).all()
        gidx[m, p, col] = trow[src_s]

    return {
        "ghat": ghat,
        "hband": hband,
        "boff": boff,
        "slot_p": slot_p,
        "gidx": gidx,
        "cnt_lane": cnt_lane,
        "node_of_rank": node_of_rank,
        "trow": trow,
    }


def _build_program(hband, boff, slot_p):
    nc = bacc.Bacc("TRN2", target_bir_lowering=False, debug=False,
                   num_devices=M)

    xT = nc.dram_tensor("xT", [IN_F, SHARD], F32, kind="ExternalInput")
    w0 = nc.dram_tensor("w0", [IN_F, HID], F32, kind="ExternalInput")
    b0 = nc.dram_tensor("b0", [HID, 1], F32, kind="ExternalInput")
    w1 = nc.dram_tensor("w1", [HID, HID], F32, kind="ExternalInput")
    b1 = nc.dram_tensor("b1", [HID, 1], F32, kind="ExternalInput")
    w2 = nc.dram_tensor("w2", [HID, NC], F32, kind="ExternalInput")
    b2r = nc.dram_tensor("b2r", [P, NC], F32, kind="ExternalInput")
    cnt = nc.dram_tensor("cnt", [P, NCHUNK], I32, kind="ExternalInput")
    gidx = nc.dram_tensor("gidx", [P, slot_p], I32, kind="ExternalInput")
    out = nc.dram_tensor("out", [SHARD, NC], F32, kind="ExternalOutput")

    LANE_F = NCHUNK * NC

    with TileContext(nc) as tc:
        with tc.tile_pool(name="const", bufs=1) as cpool, \
             tc.tile_pool(name="mlp", bufs=3) as mpool, \
             tc.tile_pool(name="psum", bufs=4, space="PSUM") as ppool, \
             tc.tile_pool(name="psum3", bufs=2, space="PSUM") as p3pool, \
             tc.tile_pool(name="gath", bufs=8) as gpool, \
             tc.tile_pool(name="ibp", bufs=4) as ibpool, \
             tc.tile_pool(name="dram", bufs=1, space="DRAM") as dpool:

            w0a_t = cpool.tile([P, HID], F32, tag="w0a")
            w0b_t = cpool.tile([P, HID], F32, tag="w0b")
            w1_t = cpool.tile([P, HID], F32, tag="w1")
            w2_t = cpool.tile([P, NC], F32, tag="w2")
            b0_t = cpool.tile([P, 1], F32, tag="b0")
            b1_t = cpool.tile([P, 1], F32, tag="b1")
            b2_t = cpool.tile([P, NC], F32, tag="b2")
            cnt_t = cpool.tile([P, NCHUNK], I32, tag="cnt")
            norm_t = cpool.tile([P, NCHUNK], F32, tag="norm")
            norm09_t = cpool.tile([P, NCHUNK], F32, tag="norm09")
            gidx_t = cpool.tile([P, slot_p], I32, tag="gidx")
            h0s_t = cpool.tile([P, LANE_F], F32, tag="h0s")
            hA_t = cpool.tile([P, LANE_F], F32, tag="hA")
            hB_t = cpool.tile([P, LANE_F], F32, tag="hB")
            hn_t = cpool.tile([P, LANE_F], F32, tag="hn")
            zrow_t = cpool.tile([1, NC], F32, tag="zrow")

            stag_t = dpool.tile([SHARD_T, NC], F32, tag="stag")

            nc.sync.dma_start(w0a_t[:, :], w0[0:P, :])
            nc.sync.dma_start(w0b_t[:, :], w0[P:IN_F, :])
            nc.sync.dma_start(w1_t[:, :], w1[:, :])
            nc.sync.dma_start(w2_t[:, :], w2[:, :])
            nc.sync.dma_start(b0_t[:, :], b0[:, :])
            nc.sync.dma_start(b1_t[:, :], b1[:, :])
            nc.sync.dma_start(b2_t[:, :], b2r[:, :])
            nc.sync.dma_start(cnt_t[:, :], cnt[:, :])
            nc.sync.dma_start(gidx_t[:, :], gidx[:, :])

            nc.vector.memset(zrow_t[:, :], 0.0)
            nc.sync.dma_start(stag_t[SHARD:SHARD_T, :], zrow_t[:, :])

            nc.vector.tensor_copy(norm_t[:, :], cnt_t[:, :])
            nc.vector.tensor_scalar_max(norm_t[:, :], norm_t[:, :], 1.0)
            nc.scalar.activation(norm_t[:, :], norm_t[:, :],
                                 mybir.ActivationFunctionType.Sqrt)
            nc.vector.reciprocal(norm_t[:, :], norm_t[:, :])
            nc.vector.tensor_scalar_mul(norm09_t[:, :], norm_t[:, :],
                                        1.0 - ALPHA)

            # ---- MLP ----
            TN = 512
            n_tiles = (SHARD + TN - 1) // TN
            for t in range(n_tiles):
                c0 = t * TN
                w = min(TN, SHARD - c0)
                x1 = mpool.tile([P, TN], F32, tag="x1")
                x2 = mpool.tile([P, TN], F32, tag="x2")
                nc.sync.dma_start(x1[:, :w], xT[0:P, c0:c0 + w])
                nc.sync.dma_start(x2[:, :w], xT[P:IN_F, c0:c0 + w])
                ps1 = ppool.tile([P, TN], F32, tag="ps")
                nc.tensor.matmul(ps1[:, :w], w0a_t[:, :], x1[:, :w],
                                 start=True, stop=False)
                nc.tensor.matmul(ps1[:, :w], w0b_t[:, :], x2[:, :w],
                                 start=False, stop=True)
                h1 = mpool.tile([P, TN], F32, tag="h1")
                nc.scalar.activation(h1[:, :w], ps1[:, :w],
                                     mybir.ActivationFunctionType.Relu,
                                     bias=b0_t[:, :])
                ps2 = ppool.tile([P, TN], F32, tag="ps")
                nc.tensor.matmul(ps2[:, :w], w1_t[:, :], h1[:, :w],
                                 start=True, stop=True)
                h2 = mpool.tile([P, TN], F32, tag="h2")
                nc.scalar.activation(h2[:, :w], ps2[:, :w],
                                     mybir.ActivationFunctionType.Relu,
                                     bias=b1_t[:, :])
                for cl in range(w // P):
                    ch = c0 // P + cl
                    ps3 = p3pool.tile([P, NC], F32, tag="ps3")
                    nc.tensor.matmul(ps3[:, :],
                                     h2[:, cl * P:(cl + 1) * P],
                                     w2_t[:, :], start=True, stop=True)
                    nc.vector.tensor_tensor(
                        out=hA_t[:, ch * NC:(ch + 1) * NC],
                        in0=ps3[:, :], in1=b2_t[:, :],
                        op=mybir.AluOpType.add)

            nc.vector.tensor_scalar_mul(h0s_t[:, :], hA_t[:, :], ALPHA)

            # ---- propagation ----
            cur, nxt = hA_t, hB_t
            stag_ap = stag_t[0:SHARD, :].rearrange("(c p) f -> p c f", p=P)
            for k in range(K_HOPS):
                nc.vector.tensor_tensor(
                    out=hn_t[:, :].rearrange("p (c f) -> p c f", f=NC),
                    in0=cur[:, :].rearrange("p (c f) -> p c f", f=NC),
                    in1=norm_t[:, :].to_broadcast([P, NCHUNK, NC]),
                    op=mybir.AluOpType.mult)
                nc.sync.dma_start(stag_ap, hn_t[:, :].rearrange(
                    "p (c f) -> p c f", f=NC))
                table_t = dpool.tile([TAB, NC], F32,
                                     addr_space="Shared", tag="table")
                nc.gpsimd.collective_compute(
                    "AllGather", mybir.AluOpType.bypass,
                    replica_groups=[list(range(M))],
                    ins=[stag_t[:, :]],
                    outs=[table_t[:, :]])
                nc.vector.memset(nxt[:, :], 0.0)
                for bi, (b0c, b1c) in enumerate(BANDS):
                    nb = b1c - b0c
                    col0 = int(boff[bi])
                    ncols = int(boff[bi + 1] - boff[bi])
                    with tc.For_i(col0, col0 + ncols, nb) as i:
                        ib = ibpool.tile([P, nb], I32, tag="ib")
                        nc.vector.tensor_copy(ib[:, :],
                                              gidx_t[:, bass.ds(i, nb)])
                        for kk in range(nb):
                            ch = b0c + kk
                            g_t = gpool.tile([P, NC], F32, tag="g")
                            nc.gpsimd.indirect_dma_start(
                                out=g_t[:, :], out_offset=None,
                                in_=table_t[:, :],
                                in_offset=IndirectOffsetOnAxis(
                                    ap=ib[:, kk:kk + 1], axis=0))
                            nc.vector.tensor_tensor(
                                out=nxt[:, ch * NC:(ch + 1) * NC],
                                in0=nxt[:, ch * NC:(ch + 1) * NC],
                                in1=g_t[:, :], op=mybir.AluOpType.add)
                nc.vector.tensor_tensor(
                    out=nxt[:, :].rearrange("p (c f) -> p c f", f=NC),
                    in0=nxt[:, :].rearrange("p (c f) -> p c f", f=NC),
                    in1=norm09_t[:, :].to_broadcast([P, NCHUNK, NC]),
                    op=mybir.AluOpType.mult)
                nc.vector.tensor_tensor(out=nxt[:, :], in0=nxt[:, :],
                                        in1=h0s_t[:, :],
                                        op=mybir.AluOpType.add)
                cur, nxt = nxt, cur

            nc.sync.dma_start(
                out[:, :].rearrange("(c p) f -> p c f", p=P),
                cur[:, :].rearrange("p (c f) -> p c f", f=NC))

    nc.finalize()
    return nc


def _make_in_maps(features, W0, b0, W1, b1, W2, b2, pre):
    in_maps = []
    b2rep = np.ascontiguousarray(
        np.broadcast_to(np.asarray(b2, dtype=np.float32).reshape(1, NC),
                        (P, NC)))
    for m in range(M):
        base = m * CORE_N
        X = np.zeros((SHARD, IN_F), dtype=np.float32)
        ranks = pre["trow"][base:base + CORE_N] - m * SHARD_T
        X[ranks] = features[base:base + CORE_N]
        in_maps.append({
            "xT": np.ascontiguousarray(X.T),
            "w0": np.ascontiguousarray(W0, dtype=np.float32),
            "b0": np.ascontiguousarray(
                np.asarray(b0, dtype=np.float32).reshape(HID, 1)),
            "w1": np.ascontiguousarray(W1, dtype=np.float32),
            "b1": np.ascontiguousarray(
                np.asarray(b1, dtype=np.float32).reshape(HID, 1)),
            "w2": np.ascontiguousarray(W2, dtype=np.float32),
            "b2r": b2rep,
            "cnt": pre["cnt_lane"][m],
            "gidx": pre["gidx"][m],
        })
    return in_maps


def kernel(features, src, dst, W0, b0, W1, b1, W2, b2, _trace=False):
    features = np.asarray(features, dtype=np.float32)
    pre = _preprocess(src, dst)
    nc = _build_program(pre["hband"], pre["boff"], pre["slot_p"])
    in_maps = _make_in_maps(features, W0, b0, W1, b1, W2, b2, pre)
    res = run_bass_kernel_spmd(nc, in_maps, core_ids=list(range(M)),
                               trace=_trace)
    out = np.zeros((N_NODES, NC), dtype=np.float32)
    for m in range(M):
        nor = pre["node_of_rank"][m]
        real = nor >= 0
        out[nor[real]] = res.results[m]["out"][real]
    if _trace:
        kernel.last_exec_time_ns = res.exec_time_ns
    return out


# revision 9
# speedup vs baseline: 1.1071x; 1.1071x over previous
"""APPNP (GNN message passing) on 8 Trainium2 NeuronCores.

Sharding (follows the hint): nodes and their segment-sums are sharded
across the 8 cores by node id (12500 each, edges partitioned by dst); the
MLP weights are replicated; each hop AllGathers every core's refreshed
hn = h*norm shard into a full per-core DRAM table, and each core gathers
hn[src] for its dst-sorted edge slots with indirect (per-partition) DMA,
accumulating on the vector engine.

Per-core layout: nodes are permuted by ascending in-degree ("rank"),
rank = chunk*128 + lane over 98 chunks. Chunk c pads every node to the
chunk max in-degree ghat[c] (cross-core max so the SPMD program is
uniform). Chunks are processed in 3 degree bands; each band runs one
Tile For_i loop over slot index j, whose body copies the j-th index
column for the band's chunks and issues one 128-descriptor indirect
gather plus one accumulate per chunk.

Host preprocessing is pure index manipulation (degree counts, sorting,
slot tables); all float math runs on device.
"""

import numpy as np

import concourse.bass as bass
import concourse.mybir as mybir
from concourse import bacc
from concourse.bass import IndirectOffsetOnAxis
from concourse.bass_utils import run_bass_kernel_spmd
from concourse.tile import TileContext

N_NODES = 100000
N_EDGES = 3200000
IN_F = 256
HID = 128
NC = 16
ALPHA = 0.1
K_HOPS = 10

M = 8                   # cores
CORE_N = N_NODES // M   # 12500 real nodes per core
P = 128                 # partitions / lanes
NCHUNK = 98             # chunks of 128 ranks
SHARD = NCHUNK * P      # 12544 ranks per core (44 dummies)
SHARD_T = SHARD + 1     # table stride per core: one extra all-zero row
SENT = SHARD            # sentinel row (core 0's zero row)
TAB = M * SHARD_T       # table rows
BANDS = [(0, 33), (33, 66), (66, 97), (97, 98)]  # degree bands

F32 = mybir.dt.float32
I32 = mybir.dt.int32


def _preprocess(src, dst):
    """Pure index-space preprocessing. Returns per-core tables + metadata."""
    src = np.asarray(src).astype(np.int64)
    dst = np.asarray(dst).astype(np.int64)
    deg = np.bincount(dst, minlength=N_NODES).astype(np.int64)

    trow = np.empty(N_NODES, dtype=np.int64)
    node_of_rank = np.empty((M, SHARD), dtype=np.int64)
    cnt_lane = np.zeros((M, P, NCHUNK), dtype=np.int32)
    gmax = np.zeros((M, NCHUNK), dtype=np.int64)

    for m in range(M):
        base = m * CORE_N
        d = deg[base:base + CORE_N]
        dpad = np.concatenate([d, np.full(SHARD - CORE_N, -1, dtype=np.int64)])
        order = np.argsort(dpad, kind="stable")   # ascending; dummies first
        real = order < CORE_N
        node_of_rank[m] = np.where(real, base + order, -1)
        trow[base + order[real]] = m * SHARD_T + np.flatnonzero(real)
        dr = np.where(real, dpad[order], 0)
        cnt_lane[m] = dr.reshape(NCHUNK, P).T
        gmax[m] = dr.reshape(NCHUNK, P).max(axis=1)

    ghat = np.maximum(gmax.max(axis=0), 1).astype(np.int64)  # per chunk
    # band height = max ghat inside the band; idx layout is j-major per band
    hband = [int(ghat[b0:b1].max()) for b0, b1 in BANDS]
    boff = np.zeros(len(BANDS) + 1, dtype=np.int64)
    for bi, ((b0, b1), h) in enumerate(zip(BANDS, hband)):
        boff[bi + 1] = boff[bi] + h * (b1 - b0)
    slot_p = int(boff[-1])

    band_of_chunk = np.empty(NCHUNK, dtype=np.int64)
    for bi, (b0, b1) in enumerate(BANDS):
        band_of_chunk[b0:b1] = bi
    boff_c = boff[band_of_chunk]
    nb_c = np.array([BANDS[band_of_chunk[c]][1] - BANDS[band_of_chunk[c]][0]
                     for c in range(NCHUNK)], dtype=np.int64)
    c0_c = np.array([BANDS[band_of_chunk[c]][0] for c in range(NCHUNK)],
                    dtype=np.int64)

    gidx = np.full((M, P, slot_p), SENT, dtype=np.int32)
    core_of_dst = dst // CORE_N
    for m in range(M):
        mask = core_of_dst == m
        s_e = src[mask]
        rho = trow[dst[mask]] - m * SHARD_T
        o2 = np.argsort(rho, kind="stable")
        rho_s = rho[o2]
        src_s = s_e[o2]
        n_e = len(rho_s)
        if n_e == 0:
            continue
        first = np.r_[0, np.flatnonzero(np.diff(rho_s)) + 1]
        run_len = np.diff(np.r_[first, n_e])
        j = np.arange(n_e) - np.repeat(first, run_len)
        c = rho_s // P
        p = rho_s % P
        col = boff_c[c] + j * nb_c[c] + (c - c0_c[c])
        assert (j < ghat[c]).all()
        gidx[m, p, col] = trow[src_s]

    return {
        "ghat": ghat,
        "hband": hband,
        "boff": boff,
        "slot_p": slot_p,
        "gidx": gidx,
        "cnt_lane": cnt_lane,
        "node_of_rank": node_of_rank,
        "trow": trow,
    }


def _build_program(hband, boff, slot_p):
    nc = bacc.Bacc("TRN2", target_bir_lowering=False, debug=False,
                   num_devices=M)

    xT = nc.dram_tensor("xT", [IN_F, SHARD], F32, kind="ExternalInput")
    w0 = nc.dram_tensor("w0", [IN_F, HID], F32, kind="ExternalInput")
    b0 = nc.dram_tensor("b0", [HID, 1], F32, kind="ExternalInput")
    w1 = nc.dram_tensor("w1", [HID, HID], F32, kind="ExternalInput")
    b1 = nc.dram_tensor("b1", [HID, 1], F32, kind="ExternalInput")
    w2 = nc.dram_tensor("w2", [HID, NC], F32, kind="ExternalInput")
    b2r = nc.dram_tensor("b2r", [P, NC], F32, kind="ExternalInput")
    cnt = nc.dram_tensor("cnt", [P, NCHUNK], I32, kind="ExternalInput")
    gidx = nc.dram_tensor("gidx", [P, slot_p], I32, kind="ExternalInput")
    out = nc.dram_tensor("out", [SHARD, NC], F32, kind="ExternalOutput")

    LANE_F = NCHUNK * NC

    with TileContext(nc) as tc:
        with tc.tile_pool(name="const", bufs=1) as cpool, \
             tc.tile_pool(name="mlp", bufs=3) as mpool, \
             tc.tile_pool(name="psum", bufs=4, space="PSUM") as ppool, \
             tc.tile_pool(name="psum3", bufs=2, space="PSUM") as p3pool, \
             tc.tile_pool(name="gath", bufs=48) as gpool, \
             tc.tile_pool(name="ibp", bufs=8) as ibpool, \
             tc.tile_pool(name="dram", bufs=1, space="DRAM") as dpool:

            w0a_t = cpool.tile([P, HID], F32, tag="w0a")
            w0b_t = cpool.tile([P, HID], F32, tag="w0b")
            w1_t = cpool.tile([P, HID], F32, tag="w1")
            w2_t = cpool.tile([P, NC], F32, tag="w2")
            b0_t = cpool.tile([P, 1], F32, tag="b0")
            b1_t = cpool.tile([P, 1], F32, tag="b1")
            b2_t = cpool.tile([P, NC], F32, tag="b2")
            cnt_t = cpool.tile([P, NCHUNK], I32, tag="cnt")
            norm_t = cpool.tile([P, NCHUNK], F32, tag="norm")
            norm09_t = cpool.tile([P, NCHUNK], F32, tag="norm09")
            gidx_t = cpool.tile([P, slot_p], I32, tag="gidx")
            h0s_t = cpool.tile([P, LANE_F], F32, tag="h0s")
            hA_t = cpool.tile([P, LANE_F], F32, tag="hA")
            hB_t = cpool.tile([P, LANE_F], F32, tag="hB")
            hn_t = cpool.tile([P, LANE_F], F32, tag="hn")
            zrow_t = cpool.tile([1, NC], F32, tag="zrow")

            stag_t = dpool.tile([SHARD_T, NC], F32, tag="stag")

            nc.sync.dma_start(w0a_t[:, :], w0[0:P, :])
            nc.sync.dma_start(w0b_t[:, :], w0[P:IN_F, :])
            nc.sync.dma_start(w1_t[:, :], w1[:, :])
            nc.sync.dma_start(w2_t[:, :], w2[:, :])
            nc.sync.dma_start(b0_t[:, :], b0[:, :])
            nc.sync.dma_start(b1_t[:, :], b1[:, :])
            nc.sync.dma_start(b2_t[:, :], b2r[:, :])
            nc.sync.dma_start(cnt_t[:, :], cnt[:, :])
            nc.sync.dma_start(gidx_t[:, :], gidx[:, :])

            nc.vector.memset(zrow_t[:, :], 0.0)
            nc.sync.dma_start(stag_t[SHARD:SHARD_T, :], zrow_t[:, :])

            nc.vector.tensor_copy(norm_t[:, :], cnt_t[:, :])
            nc.vector.tensor_scalar_max(norm_t[:, :], norm_t[:, :], 1.0)
            nc.scalar.activation(norm_t[:, :], norm_t[:, :],
                                 mybir.ActivationFunctionType.Sqrt)
            nc.vector.reciprocal(norm_t[:, :], norm_t[:, :])
            nc.vector.tensor_scalar_mul(norm09_t[:, :], norm_t[:, :],
                                        1.0 - ALPHA)

            # ---- MLP ----
            TN = 512
            n_tiles = (SHARD + TN - 1) // TN
            for t in range(n_tiles):
                c0 = t * TN
                w = min(TN, SHARD - c0)
                x1 = mpool.tile([P, TN], F32, tag="x1")
                x2 = mpool.tile([P, TN], F32, tag="x2")
                nc.sync.dma_start(x1[:, :w], xT[0:P, c0:c0 + w])
                nc.sync.dma_start(x2[:, :w], xT[P:IN_F, c0:c0 + w])
                ps1 = ppool.tile([P, TN], F32, tag="ps")
                nc.tensor.matmul(ps1[:, :w], w0a_t[:, :], x1[:, :w],
                                 start=True, stop=False)
                nc.tensor.matmul(ps1[:, :w], w0b_t[:, :], x2[:, :w],
                                 start=False, stop=True)
                h1 = mpool.tile([P, TN], F32, tag="h1")
                nc.scalar.activation(h1[:, :w], ps1[:, :w],
                                     mybir.ActivationFunctionType.Relu,
                                     bias=b0_t[:, :])
                ps2 = ppool.tile([P, TN], F32, tag="ps")
                nc.tensor.matmul(ps2[:, :w], w1_t[:, :], h1[:, :w],
                                 start=True, stop=True)
                h2 = mpool.tile([P, TN], F32, tag="h2")
                nc.scalar.activation(h2[:, :w], ps2[:, :w],
                                     mybir.ActivationFunctionType.Relu,
                                     bias=b1_t[:, :])
                for cl in range(w // P):
                    ch = c0 // P + cl
                    ps3 = p3pool.tile([P, NC], F32, tag="ps3")
                    nc.tensor.matmul(ps3[:, :],
                                     h2[:, cl * P:(cl + 1) * P],
                                     w2_t[:, :], start=True, stop=True)
                    nc.vector.tensor_tensor(
                        out=hA_t[:, ch * NC:(ch + 1) * NC],
                        in0=ps3[:, :], in1=b2_t[:, :],
                        op=mybir.AluOpType.add)

            nc.vector.tensor_scalar_mul(h0s_t[:, :], hA_t[:, :], ALPHA)

            # ---- propagation ----
            cur, nxt = hA_t, hB_t
            stag_ap = stag_t[0:SHARD, :].rearrange("(c p) f -> p c f", p=P)
            for k in range(K_HOPS):
                nc.vector.tensor_tensor(
                    out=hn_t[:, :].rearrange("p (c f) -> p c f", f=NC),
                    in0=cur[:, :].rearrange("p (c f) -> p c f", f=NC),
                    in1=norm_t[:, :].to_broadcast([P, NCHUNK, NC]),
                    op=mybir.AluOpType.mult)
                nc.sync.dma_start(stag_ap, hn_t[:, :].rearrange(
                    "p (c f) -> p c f", f=NC))
                table_t = dpool.tile([TAB, NC], F32,
                                     addr_space="Shared", tag="table")
                nc.gpsimd.collective_compute(
                    "AllGather", mybir.AluOpType.bypass,
                    replica_groups=[list(range(M))],
                    ins=[stag_t[:, :]],
                    outs=[table_t[:, :]])
                nc.vector.memset(nxt[:, :], 0.0)
                for bi, (b0c, b1c) in enumerate(BANDS):
                    nb = b1c - b0c
                    col0 = int(boff[bi])
                    ncols = int(boff[bi + 1] - boff[bi])
                    with tc.For_i(col0, col0 + ncols, nb) as i:
                        ib = ibpool.tile([P, nb], I32, tag="ib")
                        nc.vector.tensor_copy(ib[:, :],
                                              gidx_t[:, bass.ds(i, nb)])
                        for kk in range(nb):
                            ch = b0c + kk
                            g_t = gpool.tile([P, NC], F32, tag="g")
                            nc.gpsimd.indirect_dma_start(
                                out=g_t[:, :], out_offset=None,
                                in_=table_t[:, :],
                                in_offset=IndirectOffsetOnAxis(
                                    ap=ib[:, kk:kk + 1], axis=0))
                            nc.vector.tensor_tensor(
                                out=nxt[:, ch * NC:(ch + 1) * NC],
                                in0=nxt[:, ch * NC:(ch + 1) * NC],
                                in1=g_t[:, :], op=mybir.AluOpType.add)
                nc.vector.tensor_tensor(
                    out=nxt[:, :].rearrange("p (c f) -> p c f", f=NC),
                    in0=nxt[:, :].rearrange("p (c f) -> p c f", f=NC),
                    in1=norm09_t[:, :].to_broadcast([P, NCHUNK, NC]),
                    op=mybir.AluOpType.mult)
                nc.vector.tensor_tensor(out=nxt[:, :], in0=nxt[:, :],
                                        in1=h0s_t[:, :],
                                        op=mybir.AluOpType.add)
                cur, nxt = nxt, cur

            nc.sync.dma_start(
                out[:, :].rearrange("(c p) f -> p c f", p=P),
                cur[:, :].rearrange("p (c f) -> p c f", f=NC))

    nc.finalize()
    return nc


def _make_in_maps(features, W0, b0, W1, b1, W2, b2, pre):
    in_maps = []
    b2rep = np.ascontiguousarray(
        np.broadcast_to(np.asarray(b2, dtype=np.float32).reshape(1, NC),
                        (P, NC)))
    for m in range(M):
        base = m * CORE_N
        X = np.zeros((SHARD, IN_F), dtype=np.float32)
        ranks = pre["trow"][base:base + CORE_N] - m * SHARD_T
        X[ranks] = features[base:base + CORE_N]
        in_maps.append({
            "xT": np.ascontiguousarray(X.T),
            "w0": np.ascontiguousarray(W0, dtype=np.float32),
            "b0": np.ascontiguousarray(
                np.asarray(b0, dtype=np.float32).reshape(HID, 1)),
            "w1": np.ascontiguousarray(W1, dtype=np.float32),
            "b1": np.ascontiguousarray(
                np.asarray(b1, dtype=np.float32).reshape(HID, 1)),
            "w2": np.ascontiguousarray(W2, dtype=np.float32),
            "b2r": b2rep,
            "cnt": pre["cnt_lane"][m],
            "gidx": pre["gidx"][m],
        })
    return in_maps


def kernel(features, src, dst, W0, b0, W1, b1, W2, b2, _trace=False):
    features = np.asarray(features, dtype=np.float32)
    pre = _preprocess(src, dst)
    nc = _build_program(pre["hband"], pre["boff"], pre["slot_p"])
    in_maps = _make_in_maps(features, W0, b0, W1, b1, W2, b2, pre)
    res = run_bass_kernel_spmd(nc, in_maps, core_ids=list(range(M)),
                               trace=_trace)
    out = np.zeros((N_NODES, NC), dtype=np.float32)
    for m in range(M):
        nor = pre["node_of_rank"][m]
        real = nor >= 0
        out[nor[real]] = res.results[m]["out"][real]
    if _trace:
        kernel.last_exec_time_ns = res.exec_time_ns
    return out
